# revision 25
# baseline (speedup 1.0000x reference)
"""AttentiveGraphConvolution (GAT-style layer) on 8 trn2 NeuronCores.

Math (reference):
    h   = x @ W                       [N, D]
    a_s = h @ attn_self               [N, 1]
    a_n = h @ attn_neigh              [N, 1]
    e   = leaky_relu(a_s + a_n.T, 0.2)
    e   = e + NEG_INF * (1 - adj)
    out = relu(softmax(e, -1) @ h)

Reformulation (exact up to rounding), with s_ij = a_s[i] + a_n[j]:
    exp(leaky(s)) = exp(0.2 s) * max(exp(0.8 s), 1)
                  = u2_i * max(w_i * v_j, v2_j)
      w = e^{0.8 a_s},  v = e^{a_n},  v2 = e^{0.2 a_n}  (u2_i cancels in softmax)
    out_i = relu( (sum_j q_ji h3_j[:D]) / (sum_j q_ji h3_j[D]) )
      q_ji = adjT_ji * max(w_i * v_j, v2_j)     [j, i] layout (v2 folded in)
      h3_j = [h_j | 1]                          [j, D+1]  (denominator folded in)

Device work per 8-chunk group (1024 j's) per core:
    t_ji = max(w_i * v_j, v2_j)   8x DVE tensor_scalar (4x bf16 mode,
                                  per-partition AP scalars)
    q    = t * adjT               1x DVE tensor_tensor (2x bf16 mode)
    64x matmul: psum[i_blk, 0:130] += q[:, blk].T @ h3[j-chunk]  (q stationary)
The 130-wide moving operand folds the softmax denominator into the matmul
stream (col 128 = ones, col 129 = pad for 4-byte alignment); the output
lands directly in [i, d] layout; there is no second rs matmul pass.

NO COLLECTIVES: each core loads the full x (bf16, 8.4 MB) and computes the
full h3 locally -- profiling showed AllGather rendezvous + protocol cost
~70 us, far more than the extra DMA. Inputs are HOST-ROTATED per core
(node axis rolled so the core's own slab is block 0), which keeps the
program rank-independent: a_s is always read from columns [0, S).
"""

import numpy as np

N = 8192
DIN = 512
DOUT = 128
NCORES = 8
S = N // NCORES   # 1024 output rows per core
GP = 4            # adjacency j-chunks per DMA descriptor group (8 KB)
MG = 8            # j-chunks per merge group (DVE op granularity)
DEBUG = False     # add intermediate-dump outputs (set before build_nc)


def _emit(nc, tc, ctx, n, s, din, dout):
    from concourse import masks, mybir

    f32 = mybir.dt.float32
    bf16 = mybir.dt.bfloat16
    AF = mybir.ActivationFunctionType
    ALU = mybir.AluOpType

    P = 128
    jc_n = n // P        # 64 j chunks over all nodes
    kc_n = din // P      # 4 contraction chunks for x @ W
    nb = 512             # moving-dim block for phase-1 matmuls
    mg_n = jc_n // MG    # 8 merge groups
    ib_n = s // P        # 8 output row blocks
    dp = dout + 2        # h3 stride: [h | 1 | pad] -> 130 (4-byte aligned)
    XT = 2048            # x tile width (4 KB descriptors)

    adjt = nc.dram_tensor("adjt", [n, s], bf16, kind="ExternalInput")
    xt = nc.dram_tensor("xt", [din, n], bf16, kind="ExternalInput")
    wmat = nc.dram_tensor("wmat", [din, dout], bf16, kind="ExternalInput")
    att = nc.dram_tensor("att", [dout, 2], bf16, kind="ExternalInput")
    out = nc.dram_tensor("out", [s, dout], f32, kind="ExternalOutput")

    const_pool = ctx.enter_context(tc.tile_pool(name="const", bufs=1))
    ph1_pool = ctx.enter_context(tc.tile_pool(name="ph1", bufs=1))
    x_pool = ctx.enter_context(tc.tile_pool(name="x", bufs=6))
    tp_psum = ctx.enter_context(tc.tile_pool(name="tp_psum", bufs=3, space="PSUM"))
    acc_psum = ctx.enter_context(tc.tile_pool(name="acc_psum", bufs=1, space="PSUM"))
    dram_pool = ctx.enter_context(tc.tile_pool(name="dram", bufs=1, space="DRAM"))
    adj_pool = ctx.enter_context(tc.tile_pool(name="adj", bufs=2))
    t_pool = ctx.enter_context(tc.tile_pool(name="t", bufs=2))
    q_pool = ctx.enter_context(tc.tile_pool(name="q", bufs=2))
    fin_pool = ctx.enter_context(tc.tile_pool(name="fin", bufs=2))

    ident = const_pool.tile([P, P], f32, name="ident")
    masks.make_identity(nc, ident[:])
    ident_bf = const_pool.tile([P, P], bf16, name="ident_bf")
    masks.make_identity(nc, ident_bf[:])

    # PE warmup: ~5k cycles of dummy matmuls while the first x DMA lands,
    # so the HAM clock gate opens (1.2 -> 2.4 GHz) before real work.
    wu_ps = tp_psum.tile([P, P], f32, name="wu_ps", tag="tp")
    for _ in range(40):
        nc.tensor.matmul(wu_ps[:], ident[:], ident[:], start=True, stop=True)

    # ---- Phase 1: full hT = (x @ W).T and av = [a_s; a_n] for all nodes ----
    w_sb = []
    for k in range(kc_n):
        wt = ph1_pool.tile([P, dout], bf16, name="w_sb", tag=f"w_sb{k}")
        nc.sync.dma_start(wt[:], wmat[k * P:(k + 1) * P, :])
        w_sb.append(wt)
    att_sb = const_pool.tile([P, 2], bf16, name="att_sb")
    nc.sync.dma_start(att_sb[:], att[:])

    hT_sb = ph1_pool.tile([P, n], bf16, name="hT_sb")
    av_sb = ph1_pool.tile([2, n], f32, name="av_sb")  # rows: a_s, a_n
    an_dram = dram_pool.tile([n, 1], f32, name="an_dram")
    vf_sb = const_pool.tile([P, jc_n], f32, name="vf_sb")
    v2f_sb = const_pool.tile([P, jc_n], f32, name="v2f_sb")
    nv2f_sb = const_pool.tile([P, jc_n], f32, name="nv2f_sb")
    wrow_sb = ph1_pool.tile([1, s], f32, name="wrow_sb")
    wb_sb = const_pool.tile([P, s], bf16, name="wb_sb")
    h3big = ph1_pool.tile([P, jc_n * dp], bf16, name="h3big")
    ones_sb = const_pool.tile([1, P], f32, name="ones_sb")
    nc.gpsimd.memset(ones_sb[:], 1.0)

    # Bulk-load DMA emission order matters: the Sync engine issues dma_starts
    # in order, and a dma_start that depends on compute head-of-line blocks
    # later descriptor enqueues. Emit all independent bulk loads first:
    # x quarter 0 (split in halves so the first 1024 nodes land sooner), the
    # first two adjacency groups, then the remaining x quarters.
    qn = n // XT               # 4 quarters
    cq = XT // P               # 16 j-chunks per quarter
    xq = []
    for qt in range(qn):
        xh = []
        for k in range(kc_n):
            xk = x_pool.tile([P, XT], bf16, name="x_sb")
            for hf in range(2):
                fs = slice(hf * (XT // 2), (hf + 1) * (XT // 2))
                nc.sync.dma_start(
                    xk[:, fs], xt[k * P:(k + 1) * P, qt * XT:(qt + 1) * XT][:, fs])
            xh.append(xk)
        xq.append(xh)
        if qt == 0:
            pre_adj = []
            for G in range(2):
                adj_t = adj_pool.tile([P, MG * s], bf16, name="adj_t")
                for half in range(MG // GP):
                    g = G * (MG // GP) + half
                    nc.sync.dma_start(
                        adj_t[:, half * GP * s:(half + 1) * GP * s],
                        adjt[g * GP * P:(g + 1) * GP * P, :].rearrange(
                            "(p r) i -> p (r i)", r=GP),
                    )
                pre_adj.append(adj_t)

    # One 1024-node half-quarter at a time: hT -> av -> a_n transposes ->
    # vf/v2f cols; wb after the first half-quarter. Merge group G needs only
    # half-quarter G's scalars + wb, so the DVE pipeline starts early.
    for qt in range(qn):
        xh = xq[qt]
        for half in range(2):
            for bb in (half * 2, half * 2 + 1):
                b0 = qt * XT + bb * nb
                hT_ps = tp_psum.tile([P, nb], f32, name="hT_ps", tag="tp")
                for k in range(kc_n):
                    nc.tensor.matmul(
                        hT_ps[:], w_sb[k][:], xh[k][:, bb * nb:(bb + 1) * nb],
                        start=(k == 0), stop=(k == kc_n - 1),
                    )
                nc.scalar.activation(hT_sb[:, b0:b0 + nb], hT_ps[:], AF.Copy)
                av_ps = tp_psum.tile([2, nb], f32, name="av_ps", tag="tp")
                nc.tensor.matmul(
                    av_ps[:], att_sb[:], hT_sb[:, b0:b0 + nb],
                    start=True, stop=True,
                )
                nc.scalar.activation(av_sb[:, b0:b0 + nb], av_ps[:], AF.Copy)

            # transpose a_n for this half-quarter's 8 chunks via tiny PE
            # transposes (cols 2c|2c+1 = a_s|a_n), then strided-input exps
            hq = qt * 2 + half
            avT_ps = tp_psum.tile([P, 16], f32, name="avT_ps", tag="tp")
            for c in range(8):
                j = hq * 8 + c
                nc.tensor.matmul(
                    avT_ps[:, 2 * c:2 * c + 2],
                    av_sb[:, j * P:(j + 1) * P], ident[:2, :2],
                    is_transpose=True, start=True, stop=True,
                )
            csl = slice(hq * 8, (hq + 1) * 8)
            anT = avT_ps[:].rearrange("p (c two) -> p c two", two=2)[:, :, 1]
            nc.scalar.activation(vf_sb[:, csl], anT, AF.Exp, scale=1.0)
            nc.scalar.activation(v2f_sb[:, csl], anT, AF.Exp, scale=0.2)
            nc.vector.tensor_scalar(nv2f_sb[:, csl], v2f_sb[:, csl], -1.0,
                                    None, ALU.mult)

            if hq == 0:
                # wb[p, i] = exp(0.8 * a_s_local[i]); host rotation puts the
                # local slab at nodes [0, s)
                nc.scalar.activation(wrow_sb[:], av_sb[0:1, :s], AF.Exp,
                                     scale=0.8)
                for b in range(s // nb):
                    wb_ps = tp_psum.tile([P, nb], f32, name="wb_ps", tag="tp")
                    nc.tensor.matmul(
                        wb_ps[:], ones_sb[:], wrow_sb[:, b * nb:(b + 1) * nb],
                        start=True, stop=True,
                    )
                    nc.scalar.activation(wb_sb[:, b * nb:(b + 1) * nb],
                                         wb_ps[:], AF.Copy)

            # h3big chunks: h3big[p, c*130+d] = h[c*128+p, d]
            for c4 in range(hq * 2, (hq + 1) * 2):
                tr_ps = tp_psum.tile([P, 4 * P], bf16, name="tr_ps", tag="tp")
                for u in range(4):
                    c = c4 * 4 + u
                    nc.tensor.matmul(
                        tr_ps[:, u * P:(u + 1) * P], hT_sb[:, c * P:(c + 1) * P],
                        ident_bf[:], is_transpose=True, start=True, stop=True,
                    )
                dst = h3big[:, c4 * 4 * dp:(c4 + 1) * 4 * dp].rearrange(
                    "p (c d) -> p c d", c=4, d=dp)[:, :, :dout]
                nc.scalar.activation(
                    dst, tr_ps[:].rearrange("p (c d) -> p c d", c=4), AF.Copy)
            # ones in col 128 (denominator source); col 129 also set (pad)
            onecol = h3big[:, hq * 8 * dp:(hq + 1) * 8 * dp].rearrange(
                "p (c d) -> p c d", c=8)[:, :, dout:dp]
            nc.gpsimd.memset(onecol, 1.0)

    if DEBUG:
        dbg_hT = nc.dram_tensor("dbg_hT", [P, n], bf16, kind="ExternalOutput")
        nc.sync.dma_start(dbg_hT[:], hT_sb[:])
        dbg_av = nc.dram_tensor("dbg_av", [2, n], f32, kind="ExternalOutput")
        nc.sync.dma_start(dbg_av[:], av_sb[:])
        dbg_wb = nc.dram_tensor("dbg_wb", [P, s], bf16, kind="ExternalOutput")
        nc.sync.dma_start(dbg_wb[:], wb_sb[:])
        dbg_vf = nc.dram_tensor("dbg_vf", [P, jc_n], f32, kind="ExternalOutput")
        nc.sync.dma_start(dbg_vf[:], vf_sb[:])
        dbg_v2f = nc.dram_tensor("dbg_v2f", [P, jc_n], f32, kind="ExternalOutput")
        nc.sync.dma_start(dbg_v2f[:], v2f_sb[:])
        dbg_h3 = nc.dram_tensor("dbg_h3", [P, jc_n * dp], bf16,
                                kind="ExternalOutput")
        nc.sync.dma_start(dbg_h3[:], h3big[:])
        dbg_t = nc.dram_tensor("dbg_t", [P, MG * s], bf16, kind="ExternalOutput")
        dbg_q = nc.dram_tensor("dbg_q", [P, MG * s], bf16, kind="ExternalOutput")

    # ---- Phase 3: main loop over merge groups ------------------------------
    # 4 full-bank psum tiles, each holds two [i_blk, 130] accumulators.
    # NOTE: a matmul's start=True clears has_written flags for its whole PSUM
    # bank, so only the FIRST slice's first matmul may use start=True; the
    # second slice's first matmul relies on cleared flags -> overwrite.
    mm_ps = [acc_psum.tile([P, 512], f32, name=f"mm_ps{v}") for v in range(4)]

    def acc_slice(b):
        return mm_ps[b // 2][:, (b % 2) * dp:(b % 2) * dp + dp]

    for G in range(mg_n):
        if G < 2:
            adj_t = pre_adj[G]
        else:
            adj_t = adj_pool.tile([P, MG * s], bf16, name="adj_t")
            for half in range(MG // GP):
                g = G * (MG // GP) + half
                nc.sync.dma_start(
                    adj_t[:, half * GP * s:(half + 1) * GP * s],
                    adjt[g * GP * P:(g + 1) * GP * P, :].rearrange(
                        "(p r) i -> p (r i)", r=GP),
                )
        t_t = t_pool.tile([P, MG * s], bf16, name="t_t")
        for r in range(MG):
            j = G * MG + r
            if G >= 4 and r in (2, 5, 7):
                # offload to ScalarE: t = relu(v*w - v2) + v2 (2 passes);
                # ACT is idle during the later main loop
                tsc = fin_pool.tile([P, s], bf16, name="tsc", tag="tsc")
                nc.scalar.activation(
                    tsc[:], wb_sb[:], AF.Relu,
                    bias=nv2f_sb[:, j:j + 1], scale=vf_sb[:, j:j + 1],
                )
                nc.scalar.activation(
                    t_t[:, r * s:(r + 1) * s], tsc[:], AF.Identity,
                    bias=v2f_sb[:, j:j + 1],
                )
            else:
                # t = max(v_j * w_i, v2_j): per-partition AP scalars, 4x mode
                nc.vector.tensor_scalar(
                    t_t[:, r * s:(r + 1) * s], wb_sb[:],
                    vf_sb[:, j:j + 1], v2f_sb[:, j:j + 1], ALU.mult, ALU.max,
                )
        q_t = q_pool.tile([P, MG * s], bf16, name="q_t")
        if G < mg_n - 1:
            nc.vector.tensor_tensor(q_t[:], t_t[:], adj_t[:], ALU.mult)
        else:
            # split the final merge so the PE drain tail is short
            for r0, rl in ((0, 4), (4, 3), (7, 1)):
                sl2 = slice(r0 * s, (r0 + rl) * s)
                nc.vector.tensor_tensor(q_t[:, sl2], t_t[:, sl2],
                                        adj_t[:, sl2], ALU.mult)
        if DEBUG and G == 0:
            nc.sync.dma_start(dbg_t[:], t_t[:])
            nc.sync.dma_start(dbg_q[:], q_t[:])
        for r in range(MG):
            j = G * MG + r
            rhs = h3big[:, j * dp:(j + 1) * dp]
            for b in range(ib_n):
                nc.tensor.matmul(
                    acc_slice(b),
                    q_t[:, r * s + b * P:r * s + (b + 1) * P],
                    rhs,
                    start=(j == 0 and b % 2 == 0), stop=(j == jc_n - 1),
                    skip_group_check=True,
                )

    # ---- Phase 4: normalize + relu, direct [i, d] layout -------------------
    for b in range(ib_n):
        ps = acc_slice(b)
        rr_sb = fin_pool.tile([P, 1], f32, name="rr_sb", tag="rr")
        nc.vector.reciprocal(rr_sb[:], ps[:, dout:dout + 1])
        oc_sb = fin_pool.tile([P, dout], f32, name="oc_sb")
        nc.scalar.activation(oc_sb[:], ps[:, :dout], AF.Relu, scale=rr_sb[:])
        nc.sync.dma_start(out[b * P:(b + 1) * P, :], oc_sb[:])


def build_nc(n=N, s=S, din=DIN, dout=DOUT):
    from contextlib import ExitStack

    import concourse.bacc as bacc
    import concourse.tile as tile

    nc = bacc.Bacc(
        "TRN2",
        target_bir_lowering=False,
        debug=False,
        num_devices=NCORES,
    )
    with tile.TileContext(nc) as tc, ExitStack() as ctx:
        _emit(nc, tc, ctx, n, s, din, dout)
    nc.compile()
    return nc


def prep_adjt(adj_slab_T):
    """[n, s] rotated adjacency (transposed) -> bf16 with GP-row interleave."""
    import ml_dtypes

    n, s = adj_slab_T.shape
    P = 128
    g = n // (GP * P)
    a = adj_slab_T.reshape(g, GP, P, s).transpose(0, 2, 1, 3).reshape(n, s)
    return np.ascontiguousarray(a.astype(ml_dtypes.bfloat16))


def make_in_maps(x, adj, W, attn_self, attn_neigh, s=S):
    import ml_dtypes

    att = np.concatenate([attn_self, attn_neigh], axis=1).astype(
        ml_dtypes.bfloat16)
    W16 = np.ascontiguousarray(W.astype(ml_dtypes.bfloat16))
    xT16 = np.ascontiguousarray(x.T.astype(ml_dtypes.bfloat16))  # [din, n]
    in_maps = []
    for c in range(NCORES):
        o = c * s
        # rotate node axis so this core's slab comes first
        xt_c = np.ascontiguousarray(np.roll(xT16, -o, axis=1))
        adjT_c = np.roll(adj[o:o + s, :].T, -o, axis=0)  # [n(rot), s]
        in_maps.append({
            "adjt": prep_adjt(adjT_c),
            "xt": xt_c,
            "wmat": W16,
            "att": att,
        })
    return in_maps


def kernel(x, adj, W, attn_self, attn_neigh):
    from concourse.bass_utils import run_bass_kernel_spmd

    x = np.asarray(x, dtype=np.float32)
    adj = np.asarray(adj, dtype=np.float32)
    W = np.asarray(W, dtype=np.float32)
    attn_self = np.asarray(attn_self, dtype=np.float32)
    attn_neigh = np.asarray(attn_neigh, dtype=np.float32)

    nc = build_nc()
    in_maps = make_in_maps(x, adj, W, attn_self, attn_neigh)
    res = run_bass_kernel_spmd(nc, in_maps, list(range(NCORES)))
    return np.concatenate([res.results[c]["out"] for c in range(NCORES)], axis=0)


# revision 28
# speedup vs baseline: 1.0193x; 1.0193x over previous
"""AttentiveGraphConvolution (GAT-style layer) on 8 trn2 NeuronCores.

Math (reference):
    h   = x @ W                       [N, D]
    a_s = h @ attn_self               [N, 1]
    a_n = h @ attn_neigh              [N, 1]
    e   = leaky_relu(a_s + a_n.T, 0.2)
    e   = e + NEG_INF * (1 - adj)
    out = relu(softmax(e, -1) @ h)

Reformulation (exact up to rounding), with s_ij = a_s[i] + a_n[j]:
    exp(leaky(s)) = exp(0.2 s) * max(exp(0.8 s), 1)
                  = u2_i * max(w_i * v_j, v2_j)
      w = e^{0.8 a_s},  v = e^{a_n},  v2 = e^{0.2 a_n}  (u2_i cancels in softmax)
    out_i = relu( (sum_j q_ji h3_j[:D]) / (sum_j q_ji h3_j[D]) )
      q_ji = adjT_ji * max(w_i * v_j, v2_j)     [j, i] layout (v2 folded in)
      h3_j = [h_j | 1]                          [j, D+1]  (denominator folded in)

Device work per 8-chunk group (1024 j's) per core:
    t_ji = max(w_i * v_j, v2_j)   8x DVE tensor_scalar (4x bf16 mode,
                                  per-partition AP scalars)
    q    = t * adjT               1x DVE tensor_tensor (2x bf16 mode)
    64x matmul: psum[i_blk, 0:130] += q[:, blk].T @ h3[j-chunk]  (q stationary)
The 130-wide moving operand folds the softmax denominator into the matmul
stream (col 128 = ones, col 129 = pad for 4-byte alignment); the output
lands directly in [i, d] layout; there is no second rs matmul pass.

NO COLLECTIVES: each core loads the full x (bf16, 8.4 MB) and computes the
full h3 locally -- profiling showed AllGather rendezvous + protocol cost
~70 us, far more than the extra DMA. Inputs are HOST-ROTATED per core
(node axis rolled so the core's own slab is block 0), which keeps the
program rank-independent: a_s is always read from columns [0, S).
"""

import numpy as np

N = 8192
DIN = 512
DOUT = 128
NCORES = 8
S = N // NCORES   # 1024 output rows per core
GP = 4            # adjacency j-chunks per DMA descriptor group (8 KB)
MG = 8            # j-chunks per merge group (DVE op granularity)
DEBUG = False     # add intermediate-dump outputs (set before build_nc)


def _emit(nc, tc, ctx, n, s, din, dout):
    from concourse import masks, mybir

    f32 = mybir.dt.float32
    bf16 = mybir.dt.bfloat16
    AF = mybir.ActivationFunctionType
    ALU = mybir.AluOpType

    P = 128
    jc_n = n // P        # 64 j chunks over all nodes
    kc_n = din // P      # 4 contraction chunks for x @ W
    nb = 512             # moving-dim block for phase-1 matmuls
    mg_n = jc_n // MG    # 8 merge groups
    ib_n = s // P        # 8 output row blocks
    dp = dout + 2        # h3 stride: [h | 1 | pad] -> 130 (4-byte aligned)
    XT = 2048            # x tile width (4 KB descriptors)

    adjt = nc.dram_tensor("adjt", [n, s], bf16, kind="ExternalInput")
    xt = nc.dram_tensor("xt", [din, n], bf16, kind="ExternalInput")
    wmat = nc.dram_tensor("wmat", [din, dout], bf16, kind="ExternalInput")
    att = nc.dram_tensor("att", [dout, 2], bf16, kind="ExternalInput")
    # output stored transposed-blocked: out[p, b*128+d] = row (b*128+p), col d
    out = nc.dram_tensor("out", [128, s], f32, kind="ExternalOutput")

    const_pool = ctx.enter_context(tc.tile_pool(name="const", bufs=1))
    ph1_pool = ctx.enter_context(tc.tile_pool(name="ph1", bufs=1))
    x_pool = ctx.enter_context(tc.tile_pool(name="x", bufs=6))
    tp_psum = ctx.enter_context(tc.tile_pool(name="tp_psum", bufs=3, space="PSUM"))
    acc_psum = ctx.enter_context(tc.tile_pool(name="acc_psum", bufs=1, space="PSUM"))
    dram_pool = ctx.enter_context(tc.tile_pool(name="dram", bufs=1, space="DRAM"))
    adj_pool = ctx.enter_context(tc.tile_pool(name="adj", bufs=2))
    t_pool = ctx.enter_context(tc.tile_pool(name="t", bufs=2))
    q_pool = ctx.enter_context(tc.tile_pool(name="q", bufs=2))
    fin_pool = ctx.enter_context(tc.tile_pool(name="fin", bufs=4))

    ident = const_pool.tile([P, P], f32, name="ident")
    masks.make_identity(nc, ident[:])
    ident_bf = const_pool.tile([P, P], bf16, name="ident_bf")
    masks.make_identity(nc, ident_bf[:])

    # PE warmup: ~5k cycles of dummy matmuls while the first x DMA lands,
    # so the HAM clock gate opens (1.2 -> 2.4 GHz) before real work.
    wu_ps = tp_psum.tile([P, P], f32, name="wu_ps", tag="tp")
    for _ in range(40):
        nc.tensor.matmul(wu_ps[:], ident[:], ident[:], start=True, stop=True)

    # ---- Phase 1: full hT = (x @ W).T and av = [a_s; a_n] for all nodes ----
    w_sb = []
    for k in range(kc_n):
        wt = ph1_pool.tile([P, dout], bf16, name="w_sb", tag=f"w_sb{k}")
        nc.sync.dma_start(wt[:], wmat[k * P:(k + 1) * P, :])
        w_sb.append(wt)
    att_sb = const_pool.tile([P, 2], bf16, name="att_sb")
    nc.sync.dma_start(att_sb[:], att[:])

    hT_sb = ph1_pool.tile([P, n], bf16, name="hT_sb")
    av_sb = ph1_pool.tile([2, n], f32, name="av_sb")  # rows: a_s, a_n
    an_dram = dram_pool.tile([n, 1], f32, name="an_dram")
    vf_sb = const_pool.tile([P, jc_n], f32, name="vf_sb")
    v2f_sb = const_pool.tile([P, jc_n], f32, name="v2f_sb")
    wrow_sb = ph1_pool.tile([1, s], f32, name="wrow_sb")
    wb_sb = const_pool.tile([P, s], bf16, name="wb_sb")
    h3big = ph1_pool.tile([P, jc_n * dp], bf16, name="h3big")
    ones_sb = const_pool.tile([1, P], f32, name="ones_sb")
    nc.gpsimd.memset(ones_sb[:], 1.0)

    # Bulk-load DMA emission order matters: the Sync engine issues dma_starts
    # in order, and a dma_start that depends on compute head-of-line blocks
    # later descriptor enqueues. Emit all independent bulk loads first:
    # x quarter 0 (split in halves so the first 1024 nodes land sooner), the
    # first two adjacency groups, then the remaining x quarters.
    qn = n // XT               # 4 quarters
    cq = XT // P               # 16 j-chunks per quarter
    xq = []
    for qt in range(qn):
        xh = []
        for k in range(kc_n):
            xk = x_pool.tile([P, XT], bf16, name="x_sb")
            for hf in range(2):
                fs = slice(hf * (XT // 2), (hf + 1) * (XT // 2))
                nc.sync.dma_start(
                    xk[:, fs], xt[k * P:(k + 1) * P, qt * XT:(qt + 1) * XT][:, fs])
            xh.append(xk)
        xq.append(xh)
        if qt == 0:
            pre_adj = []
            for G in range(2):
                adj_t = adj_pool.tile([P, MG * s], bf16, name="adj_t")
                for half in range(MG // GP):
                    g = G * (MG // GP) + half
                    nc.sync.dma_start(
                        adj_t[:, half * GP * s:(half + 1) * GP * s],
                        adjt[g * GP * P:(g + 1) * GP * P, :].rearrange(
                            "(p r) i -> p (r i)", r=GP),
                    )
                pre_adj.append(adj_t)

    # One 1024-node half-quarter at a time: hT -> av -> a_n transposes ->
    # vf/v2f cols; wb after the first half-quarter. Merge group G needs only
    # half-quarter G's scalars + wb, so the DVE pipeline starts early.
    for qt in range(qn):
        xh = xq[qt]
        for half in range(2):
            for bb in (half * 2, half * 2 + 1):
                b0 = qt * XT + bb * nb
                hT_ps = tp_psum.tile([P, nb], f32, name="hT_ps", tag="tp")
                for k in range(kc_n):
                    nc.tensor.matmul(
                        hT_ps[:], w_sb[k][:], xh[k][:, bb * nb:(bb + 1) * nb],
                        start=(k == 0), stop=(k == kc_n - 1),
                    )
                nc.scalar.activation(hT_sb[:, b0:b0 + nb], hT_ps[:], AF.Copy)
                av_ps = tp_psum.tile([2, nb], f32, name="av_ps", tag="tp")
                nc.tensor.matmul(
                    av_ps[:], att_sb[:], hT_sb[:, b0:b0 + nb],
                    start=True, stop=True,
                )
                nc.scalar.activation(av_sb[:, b0:b0 + nb], av_ps[:], AF.Copy)

            # transpose a_n for this half-quarter's 8 chunks via tiny PE
            # transposes (cols 2c|2c+1 = a_s|a_n), then strided-input exps
            hq = qt * 2 + half
            avT_ps = tp_psum.tile([P, 16], f32, name="avT_ps", tag="tp")
            for c in range(8):
                j = hq * 8 + c
                nc.tensor.matmul(
                    avT_ps[:, 2 * c:2 * c + 2],
                    av_sb[:, j * P:(j + 1) * P], ident[:2, :2],
                    is_transpose=True, start=True, stop=True,
                )
            csl = slice(hq * 8, (hq + 1) * 8)
            anT = avT_ps[:].rearrange("p (c two) -> p c two", two=2)[:, :, 1]
            nc.scalar.activation(vf_sb[:, csl], anT, AF.Exp, scale=1.0)
            nc.scalar.activation(v2f_sb[:, csl], anT, AF.Exp, scale=0.2)

            if hq == 0:
                # wb[p, i] = exp(0.8 * a_s_local[i]); host rotation puts the
                # local slab at nodes [0, s)
                nc.scalar.activation(wrow_sb[:], av_sb[0:1, :s], AF.Exp,
                                     scale=0.8)
                for b in range(s // nb):
                    wb_ps = tp_psum.tile([P, nb], f32, name="wb_ps", tag="tp")
                    nc.tensor.matmul(
                        wb_ps[:], ones_sb[:], wrow_sb[:, b * nb:(b + 1) * nb],
                        start=True, stop=True,
                    )
                    nc.scalar.activation(wb_sb[:, b * nb:(b + 1) * nb],
                                         wb_ps[:], AF.Copy)

            # h3big chunks: h3big[p, c*130+d] = h[c*128+p, d]
            for c4 in range(hq * 2, (hq + 1) * 2):
                tr_ps = tp_psum.tile([P, 4 * P], bf16, name="tr_ps", tag="tp")
                for u in range(4):
                    c = c4 * 4 + u
                    nc.tensor.matmul(
                        tr_ps[:, u * P:(u + 1) * P], hT_sb[:, c * P:(c + 1) * P],
                        ident_bf[:], is_transpose=True, start=True, stop=True,
                    )
                dst = h3big[:, c4 * 4 * dp:(c4 + 1) * 4 * dp].rearrange(
                    "p (c d) -> p c d", c=4, d=dp)[:, :, :dout]
                nc.scalar.activation(
                    dst, tr_ps[:].rearrange("p (c d) -> p c d", c=4), AF.Copy)
            # ones in col 128 (denominator source); col 129 also set (pad)
            onecol = h3big[:, hq * 8 * dp:(hq + 1) * 8 * dp].rearrange(
                "p (c d) -> p c d", c=8)[:, :, dout:dp]
            nc.gpsimd.memset(onecol, 1.0)

    if DEBUG:
        dbg_hT = nc.dram_tensor("dbg_hT", [P, n], bf16, kind="ExternalOutput")
        nc.sync.dma_start(dbg_hT[:], hT_sb[:])
        dbg_av = nc.dram_tensor("dbg_av", [2, n], f32, kind="ExternalOutput")
        nc.sync.dma_start(dbg_av[:], av_sb[:])
        dbg_wb = nc.dram_tensor("dbg_wb", [P, s], bf16, kind="ExternalOutput")
        nc.sync.dma_start(dbg_wb[:], wb_sb[:])
        dbg_vf = nc.dram_tensor("dbg_vf", [P, jc_n], f32, kind="ExternalOutput")
        nc.sync.dma_start(dbg_vf[:], vf_sb[:])
        dbg_v2f = nc.dram_tensor("dbg_v2f", [P, jc_n], f32, kind="ExternalOutput")
        nc.sync.dma_start(dbg_v2f[:], v2f_sb[:])
        dbg_h3 = nc.dram_tensor("dbg_h3", [P, jc_n * dp], bf16,
                                kind="ExternalOutput")
        nc.sync.dma_start(dbg_h3[:], h3big[:])
        dbg_t = nc.dram_tensor("dbg_t", [P, MG * s], bf16, kind="ExternalOutput")
        dbg_q = nc.dram_tensor("dbg_q", [P, MG * s], bf16, kind="ExternalOutput")

    # ---- Phase 3: main loop over merge groups ------------------------------
    # 4 full-bank psum tiles, each holds two [i_blk, 130] accumulators.
    # NOTE: a matmul's start=True clears has_written flags for its whole PSUM
    # bank, so only the FIRST slice's first matmul may use start=True; the
    # second slice's first matmul relies on cleared flags -> overwrite.
    mm_ps = [acc_psum.tile([P, 512], f32, name=f"mm_ps{v}") for v in range(4)]

    def acc_slice(b):
        return mm_ps[b // 2][:, (b % 2) * dp:(b % 2) * dp + dp]

    for G in range(mg_n):
        if G < 2:
            adj_t = pre_adj[G]
        else:
            adj_t = adj_pool.tile([P, MG * s], bf16, name="adj_t")
            for half in range(MG // GP):
                g = G * (MG // GP) + half
                nc.sync.dma_start(
                    adj_t[:, half * GP * s:(half + 1) * GP * s],
                    adjt[g * GP * P:(g + 1) * GP * P, :].rearrange(
                        "(p r) i -> p (r i)", r=GP),
                )
        t_t = t_pool.tile([P, MG * s], bf16, name="t_t")
        for r in range(MG):
            j = G * MG + r
            # t = max(v_j * w_i, v2_j): per-partition AP scalars, 4x mode
            nc.vector.tensor_scalar(
                t_t[:, r * s:(r + 1) * s], wb_sb[:],
                vf_sb[:, j:j + 1], v2f_sb[:, j:j + 1], ALU.mult, ALU.max,
            )
        q_t = q_pool.tile([P, MG * s], bf16, name="q_t")
        if G < mg_n - 1:
            nc.vector.tensor_tensor(q_t[:], t_t[:], adj_t[:], ALU.mult)
        else:
            # split the final merge so the PE drain tail is short
            for r0, rl in ((0, 4), (4, 3), (7, 1)):
                sl2 = slice(r0 * s, (r0 + rl) * s)
                nc.vector.tensor_tensor(q_t[:, sl2], t_t[:, sl2],
                                        adj_t[:, sl2], ALU.mult)
        if DEBUG and G == 0:
            nc.sync.dma_start(dbg_t[:], t_t[:])
            nc.sync.dma_start(dbg_q[:], q_t[:])
        for r in range(MG):
            j = G * MG + r
            rhs = h3big[:, j * dp:(j + 1) * dp]
            for b in range(ib_n):
                nc.tensor.matmul(
                    acc_slice(b),
                    q_t[:, r * s + b * P:r * s + (b + 1) * P],
                    rhs,
                    start=(j == 0 and b % 2 == 0), stop=(j == jc_n - 1),
                    skip_group_check=True,
                )

    # ---- Phase 4: normalize + relu, direct [i, d] layout -------------------
    oc_big = ph1_pool.tile([P, s], f32, name="oc_big")
    for b in range(ib_n):
        ps = acc_slice(b)
        rr_sb = fin_pool.tile([P, 1], f32, name="rr_sb", tag="rr")
        nc.vector.reciprocal(rr_sb[:], ps[:, dout:dout + 1])
        nc.scalar.activation(oc_big[:, b * dout:(b + 1) * dout], ps[:, :dout],
                             AF.Relu, scale=rr_sb[:])
    nc.sync.dma_start(out[:], oc_big[:])


def build_nc(n=N, s=S, din=DIN, dout=DOUT):
    from contextlib import ExitStack

    import concourse.bacc as bacc
    import concourse.tile as tile

    nc = bacc.Bacc(
        "TRN2",
        target_bir_lowering=False,
        debug=False,
        num_devices=NCORES,
    )
    with tile.TileContext(nc) as tc, ExitStack() as ctx:
        _emit(nc, tc, ctx, n, s, din, dout)
    nc.compile()
    return nc


def prep_adjt(adj_slab_T):
    """[n, s] rotated adjacency (transposed) -> bf16 with GP-row interleave."""
    import ml_dtypes

    n, s = adj_slab_T.shape
    P = 128
    g = n // (GP * P)
    a = adj_slab_T.reshape(g, GP, P, s).transpose(0, 2, 1, 3).reshape(n, s)
    return np.ascontiguousarray(a.astype(ml_dtypes.bfloat16))


def make_in_maps(x, adj, W, attn_self, attn_neigh, s=S):
    import ml_dtypes

    att = np.concatenate([attn_self, attn_neigh], axis=1).astype(
        ml_dtypes.bfloat16)
    W16 = np.ascontiguousarray(W.astype(ml_dtypes.bfloat16))
    xT16 = np.ascontiguousarray(x.T.astype(ml_dtypes.bfloat16))  # [din, n]
    in_maps = []
    for c in range(NCORES):
        o = c * s
        # rotate node axis so this core's slab comes first
        xt_c = np.ascontiguousarray(np.roll(xT16, -o, axis=1))
        adjT_c = np.roll(adj[o:o + s, :].T, -o, axis=0)  # [n(rot), s]
        in_maps.append({
            "adjt": prep_adjt(adjT_c),
            "xt": xt_c,
            "wmat": W16,
            "att": att,
        })
    return in_maps


def kernel(x, adj, W, attn_self, attn_neigh):
    from concourse.bass_utils import run_bass_kernel_spmd

    x = np.asarray(x, dtype=np.float32)
    adj = np.asarray(adj, dtype=np.float32)
    W = np.asarray(W, dtype=np.float32)
    attn_self = np.asarray(attn_self, dtype=np.float32)
    attn_neigh = np.asarray(attn_neigh, dtype=np.float32)

    nc = build_nc()
    in_maps = make_in_maps(x, adj, W, attn_self, attn_neigh)
    res = run_bass_kernel_spmd(nc, in_maps, list(range(NCORES)))
    outs = []
    for c in range(NCORES):
        a = np.asarray(res.results[c]["out"])  # [128, 1024]
        outs.append(a.reshape(128, 8, 128).transpose(1, 0, 2).reshape(S, DOUT))
    return np.concatenate(outs, axis=0)


# revision 29
# speedup vs baseline: 1.0340x; 1.0145x over previous
"""AttentiveGraphConvolution (GAT-style layer) on 8 trn2 NeuronCores.

Math (reference):
    h   = x @ W                       [N, D]
    a_s = h @ attn_self               [N, 1]
    a_n = h @ attn_neigh              [N, 1]
    e   = leaky_relu(a_s + a_n.T, 0.2)
    e   = e + NEG_INF * (1 - adj)
    out = relu(softmax(e, -1) @ h)

Reformulation (exact up to rounding), with s_ij = a_s[i] + a_n[j]:
    exp(leaky(s)) = exp(0.2 s) * max(exp(0.8 s), 1)
                  = u2_i * max(w_i * v_j, v2_j)
      w = e^{0.8 a_s},  v = e^{a_n},  v2 = e^{0.2 a_n}  (u2_i cancels in softmax)
    out_i = relu( (sum_j q_ji h3_j[:D]) / (sum_j q_ji h3_j[D]) )
      q_ji = adjT_ji * max(w_i * v_j, v2_j)     [j, i] layout (v2 folded in)
      h3_j = [h_j | 1]                          [j, D+1]  (denominator folded in)

Device work per 8-chunk group (1024 j's) per core:
    t_ji = max(w_i * v_j, v2_j)   8x DVE tensor_scalar (4x bf16 mode,
                                  per-partition AP scalars)
    q    = t * adjT               1x DVE tensor_tensor (2x bf16 mode)
    64x matmul: psum[i_blk, 0:130] += q[:, blk].T @ h3[j-chunk]  (q stationary)
The 130-wide moving operand folds the softmax denominator into the matmul
stream (col 128 = ones, col 129 = pad for 4-byte alignment); the output
lands directly in [i, d] layout; there is no second rs matmul pass.

NO COLLECTIVES: each core loads the full x (bf16, 8.4 MB) and computes the
full h3 locally -- profiling showed AllGather rendezvous + protocol cost
~70 us, far more than the extra DMA. Inputs are HOST-ROTATED per core
(node axis rolled so the core's own slab is block 0), which keeps the
program rank-independent: a_s is always read from columns [0, S).
"""

import numpy as np

N = 8192
DIN = 512
DOUT = 128
NCORES = 8
S = N // NCORES   # 1024 output rows per core
GP = 4            # adjacency j-chunks per DMA descriptor group (8 KB)
MG = 8            # j-chunks per merge group (DVE op granularity)
DEBUG = False     # add intermediate-dump outputs (set before build_nc)


def _emit(nc, tc, ctx, n, s, din, dout):
    from concourse import masks, mybir

    f32 = mybir.dt.float32
    bf16 = mybir.dt.bfloat16
    AF = mybir.ActivationFunctionType
    ALU = mybir.AluOpType

    P = 128
    jc_n = n // P        # 64 j chunks over all nodes
    kc_n = din // P      # 4 contraction chunks for x @ W
    nb = 512             # moving-dim block for phase-1 matmuls
    mg_n = jc_n // MG    # 8 merge groups
    ib_n = s // P        # 8 output row blocks
    dp = dout + 2        # h3 stride: [h | 1 | pad] -> 130 (4-byte aligned)
    XT = 2048            # x tile width (4 KB descriptors)

    adjt = nc.dram_tensor("adjt", [n, s], bf16, kind="ExternalInput")
    xt = nc.dram_tensor("xt", [din, n], bf16, kind="ExternalInput")
    wmat = nc.dram_tensor("wmat", [din, dout], bf16, kind="ExternalInput")
    att = nc.dram_tensor("att", [dout, 2], bf16, kind="ExternalInput")
    # output stored transposed-blocked: out[p, b*128+d] = row (b*128+p), col d
    out = nc.dram_tensor("out", [128, s], f32, kind="ExternalOutput")

    const_pool = ctx.enter_context(tc.tile_pool(name="const", bufs=1))
    ph1_pool = ctx.enter_context(tc.tile_pool(name="ph1", bufs=1))
    x_pool = ctx.enter_context(tc.tile_pool(name="x", bufs=4))
    tp_psum = ctx.enter_context(tc.tile_pool(name="tp_psum", bufs=3, space="PSUM"))
    acc_psum = ctx.enter_context(tc.tile_pool(name="acc_psum", bufs=1, space="PSUM"))
    dram_pool = ctx.enter_context(tc.tile_pool(name="dram", bufs=1, space="DRAM"))
    adj_pool = ctx.enter_context(tc.tile_pool(name="adj", bufs=3))
    t_pool = ctx.enter_context(tc.tile_pool(name="t", bufs=2))
    q_pool = ctx.enter_context(tc.tile_pool(name="q", bufs=2))
    fin_pool = ctx.enter_context(tc.tile_pool(name="fin", bufs=4))

    ident = const_pool.tile([P, P], f32, name="ident")
    masks.make_identity(nc, ident[:])
    ident_bf = const_pool.tile([P, P], bf16, name="ident_bf")
    masks.make_identity(nc, ident_bf[:])

    # PE warmup: ~5k cycles of dummy matmuls while the first x DMA lands,
    # so the HAM clock gate opens (1.2 -> 2.4 GHz) before real work.
    wu_ps = tp_psum.tile([P, P], f32, name="wu_ps", tag="tp")
    for _ in range(40):
        nc.tensor.matmul(wu_ps[:], ident[:], ident[:], start=True, stop=True)

    # ---- Phase 1: full hT = (x @ W).T and av = [a_s; a_n] for all nodes ----
    w_sb = []
    for k in range(kc_n):
        wt = ph1_pool.tile([P, dout], bf16, name="w_sb", tag=f"w_sb{k}")
        nc.sync.dma_start(wt[:], wmat[k * P:(k + 1) * P, :])
        w_sb.append(wt)
    att_sb = const_pool.tile([P, 2], bf16, name="att_sb")
    nc.sync.dma_start(att_sb[:], att[:])

    hT_sb = ph1_pool.tile([P, n], bf16, name="hT_sb")
    av_sb = ph1_pool.tile([2, n], f32, name="av_sb")  # rows: a_s, a_n
    an_dram = dram_pool.tile([n, 1], f32, name="an_dram")
    vf_sb = const_pool.tile([P, jc_n], f32, name="vf_sb")
    v2f_sb = const_pool.tile([P, jc_n], f32, name="v2f_sb")
    wrow_sb = ph1_pool.tile([1, s], f32, name="wrow_sb")
    wb_sb = const_pool.tile([P, s], bf16, name="wb_sb")
    h3big = ph1_pool.tile([P, jc_n * dp], bf16, name="h3big")
    ones_sb = const_pool.tile([1, P], f32, name="ones_sb")
    nc.gpsimd.memset(ones_sb[:], 1.0)

    # Bulk-load DMA emission order matters: the Sync engine issues dma_starts
    # in order, and a dma_start that depends on compute head-of-line blocks
    # later descriptor enqueues. Emit all independent bulk loads first:
    # x quarter 0 (split in halves so the first 1024 nodes land sooner), the
    # first two adjacency groups, then the remaining x quarters.
    qn = n // XT               # 4 quarters
    cq = XT // P               # 16 j-chunks per quarter
    xq = []
    for qt in range(qn):
        xh = []
        for k in range(kc_n):
            xk = x_pool.tile([P, XT], bf16, name="x_sb")
            for hf in range(2):
                fs = slice(hf * (XT // 2), (hf + 1) * (XT // 2))
                nc.sync.dma_start(
                    xk[:, fs], xt[k * P:(k + 1) * P, qt * XT:(qt + 1) * XT][:, fs])
            xh.append(xk)
        xq.append(xh)
        if qt == 0:
            pre_adj = []
            for G in range(2):
                adj_t = adj_pool.tile([P, MG * s], bf16, name="adj_t")
                for half in range(MG // GP):
                    g = G * (MG // GP) + half
                    nc.sync.dma_start(
                        adj_t[:, half * GP * s:(half + 1) * GP * s],
                        adjt[g * GP * P:(g + 1) * GP * P, :].rearrange(
                            "(p r) i -> p (r i)", r=GP),
                    )
                pre_adj.append(adj_t)

    # One 1024-node half-quarter at a time: hT -> av -> a_n transposes ->
    # vf/v2f cols; wb after the first half-quarter. Merge group G needs only
    # half-quarter G's scalars + wb, so the DVE pipeline starts early.
    for qt in range(qn):
        xh = xq[qt]
        for half in range(2):
            for bb in (half * 2, half * 2 + 1):
                b0 = qt * XT + bb * nb
                hT_ps = tp_psum.tile([P, nb], f32, name="hT_ps", tag="tp")
                for k in range(kc_n):
                    nc.tensor.matmul(
                        hT_ps[:], w_sb[k][:], xh[k][:, bb * nb:(bb + 1) * nb],
                        start=(k == 0), stop=(k == kc_n - 1),
                    )
                nc.scalar.activation(hT_sb[:, b0:b0 + nb], hT_ps[:], AF.Copy)
                av_ps = tp_psum.tile([2, nb], f32, name="av_ps", tag="tp")
                nc.tensor.matmul(
                    av_ps[:], att_sb[:], hT_sb[:, b0:b0 + nb],
                    start=True, stop=True,
                )
                nc.scalar.activation(av_sb[:, b0:b0 + nb], av_ps[:], AF.Copy)

            # transpose a_n for this half-quarter's 8 chunks via tiny PE
            # transposes (cols 2c|2c+1 = a_s|a_n), then strided-input exps
            hq = qt * 2 + half
            avT_ps = tp_psum.tile([P, 16], f32, name="avT_ps", tag="tp")
            for c in range(8):
                j = hq * 8 + c
                nc.tensor.matmul(
                    avT_ps[:, 2 * c:2 * c + 2],
                    av_sb[:, j * P:(j + 1) * P], ident[:2, :2],
                    is_transpose=True, start=True, stop=True,
                )
            csl = slice(hq * 8, (hq + 1) * 8)
            anT = avT_ps[:].rearrange("p (c two) -> p c two", two=2)[:, :, 1]
            nc.scalar.activation(vf_sb[:, csl], anT, AF.Exp, scale=1.0)
            nc.scalar.activation(v2f_sb[:, csl], anT, AF.Exp, scale=0.2)

            if hq == 0:
                # wb[p, i] = exp(0.8 * a_s_local[i]); host rotation puts the
                # local slab at nodes [0, s). Outer-broadcast the raw a_s row
                # on the PE, then Exp folds into the psum->sbuf copy.
                for b in range(s // nb):
                    wb_ps = tp_psum.tile([P, nb], f32, name="wb_ps", tag="tp")
                    nc.tensor.matmul(
                        wb_ps[:], ones_sb[:], av_sb[0:1, b * nb:(b + 1) * nb],
                        start=True, stop=True,
                    )
                    nc.scalar.activation(wb_sb[:, b * nb:(b + 1) * nb],
                                         wb_ps[:], AF.Exp, scale=0.8)

            # h3big chunks: h3big[p, c*130+d] = h[c*128+p, d]
            for c4 in range(hq * 2, (hq + 1) * 2):
                tr_ps = tp_psum.tile([P, 4 * P], bf16, name="tr_ps", tag="tp")
                for u in range(4):
                    c = c4 * 4 + u
                    nc.tensor.matmul(
                        tr_ps[:, u * P:(u + 1) * P], hT_sb[:, c * P:(c + 1) * P],
                        ident_bf[:], is_transpose=True, start=True, stop=True,
                    )
                dst = h3big[:, c4 * 4 * dp:(c4 + 1) * 4 * dp].rearrange(
                    "p (c d) -> p c d", c=4, d=dp)[:, :, :dout]
                nc.scalar.activation(
                    dst, tr_ps[:].rearrange("p (c d) -> p c d", c=4), AF.Copy)
            # ones in col 128 (denominator source); col 129 also set (pad)
            onecol = h3big[:, hq * 8 * dp:(hq + 1) * 8 * dp].rearrange(
                "p (c d) -> p c d", c=8)[:, :, dout:dp]
            nc.gpsimd.memset(onecol, 1.0)

    if DEBUG:
        dbg_hT = nc.dram_tensor("dbg_hT", [P, n], bf16, kind="ExternalOutput")
        nc.sync.dma_start(dbg_hT[:], hT_sb[:])
        dbg_av = nc.dram_tensor("dbg_av", [2, n], f32, kind="ExternalOutput")
        nc.sync.dma_start(dbg_av[:], av_sb[:])
        dbg_wb = nc.dram_tensor("dbg_wb", [P, s], bf16, kind="ExternalOutput")
        nc.sync.dma_start(dbg_wb[:], wb_sb[:])
        dbg_vf = nc.dram_tensor("dbg_vf", [P, jc_n], f32, kind="ExternalOutput")
        nc.sync.dma_start(dbg_vf[:], vf_sb[:])
        dbg_v2f = nc.dram_tensor("dbg_v2f", [P, jc_n], f32, kind="ExternalOutput")
        nc.sync.dma_start(dbg_v2f[:], v2f_sb[:])
        dbg_h3 = nc.dram_tensor("dbg_h3", [P, jc_n * dp], bf16,
                                kind="ExternalOutput")
        nc.sync.dma_start(dbg_h3[:], h3big[:])
        dbg_t = nc.dram_tensor("dbg_t", [P, MG * s], bf16, kind="ExternalOutput")
        dbg_q = nc.dram_tensor("dbg_q", [P, MG * s], bf16, kind="ExternalOutput")

    # ---- Phase 3: main loop over merge groups ------------------------------
    # 4 full-bank psum tiles, each holds two [i_blk, 130] accumulators.
    # NOTE: a matmul's start=True clears has_written flags for its whole PSUM
    # bank, so only the FIRST slice's first matmul may use start=True; the
    # second slice's first matmul relies on cleared flags -> overwrite.
    mm_ps = [acc_psum.tile([P, 512], f32, name=f"mm_ps{v}") for v in range(4)]

    def acc_slice(b):
        return mm_ps[b // 2][:, (b % 2) * dp:(b % 2) * dp + dp]

    for G in range(mg_n):
        if G < 2:
            adj_t = pre_adj[G]
        else:
            adj_t = adj_pool.tile([P, MG * s], bf16, name="adj_t")
            for half in range(MG // GP):
                g = G * (MG // GP) + half
                nc.sync.dma_start(
                    adj_t[:, half * GP * s:(half + 1) * GP * s],
                    adjt[g * GP * P:(g + 1) * GP * P, :].rearrange(
                        "(p r) i -> p (r i)", r=GP),
                )
        t_t = t_pool.tile([P, MG * s], bf16, name="t_t")
        for r in range(MG):
            j = G * MG + r
            # t = max(v_j * w_i, v2_j): per-partition AP scalars, 4x mode
            nc.vector.tensor_scalar(
                t_t[:, r * s:(r + 1) * s], wb_sb[:],
                vf_sb[:, j:j + 1], v2f_sb[:, j:j + 1], ALU.mult, ALU.max,
            )
        q_t = q_pool.tile([P, MG * s], bf16, name="q_t")
        if G < mg_n - 1:
            nc.vector.tensor_tensor(q_t[:], t_t[:], adj_t[:], ALU.mult)
        else:
            # split the final merge so the PE drain tail is short
            for r0, rl in ((0, 4), (4, 3), (7, 1)):
                sl2 = slice(r0 * s, (r0 + rl) * s)
                nc.vector.tensor_tensor(q_t[:, sl2], t_t[:, sl2],
                                        adj_t[:, sl2], ALU.mult)
        if DEBUG and G == 0:
            nc.sync.dma_start(dbg_t[:], t_t[:])
            nc.sync.dma_start(dbg_q[:], q_t[:])
        for r in range(MG):
            j = G * MG + r
            rhs = h3big[:, j * dp:(j + 1) * dp]
            for b in range(ib_n):
                nc.tensor.matmul(
                    acc_slice(b),
                    q_t[:, r * s + b * P:r * s + (b + 1) * P],
                    rhs,
                    start=(j == 0 and b % 2 == 0), stop=(j == jc_n - 1),
                    skip_group_check=True,
                )

    # ---- Phase 4: normalize + relu, direct [i, d] layout -------------------
    oc_big = ph1_pool.tile([P, s], f32, name="oc_big")
    for b in range(ib_n):
        ps = acc_slice(b)
        rr_sb = fin_pool.tile([P, 1], f32, name="rr_sb", tag="rr")
        nc.vector.reciprocal(rr_sb[:], ps[:, dout:dout + 1])
        nc.scalar.activation(oc_big[:, b * dout:(b + 1) * dout], ps[:, :dout],
                             AF.Relu, scale=rr_sb[:])
    nc.sync.dma_start(out[:], oc_big[:])


def build_nc(n=N, s=S, din=DIN, dout=DOUT):
    from contextlib import ExitStack

    import concourse.bacc as bacc
    import concourse.tile as tile

    nc = bacc.Bacc(
        "TRN2",
        target_bir_lowering=False,
        debug=False,
        num_devices=NCORES,
    )
    with tile.TileContext(nc) as tc, ExitStack() as ctx:
        _emit(nc, tc, ctx, n, s, din, dout)
    nc.compile()
    return nc


def prep_adjt(adj_slab_T):
    """[n, s] rotated adjacency (transposed) -> bf16 with GP-row interleave."""
    import ml_dtypes

    n, s = adj_slab_T.shape
    P = 128
    g = n // (GP * P)
    a = adj_slab_T.reshape(g, GP, P, s).transpose(0, 2, 1, 3).reshape(n, s)
    return np.ascontiguousarray(a.astype(ml_dtypes.bfloat16))


def make_in_maps(x, adj, W, attn_self, attn_neigh, s=S):
    import ml_dtypes

    att = np.concatenate([attn_self, attn_neigh], axis=1).astype(
        ml_dtypes.bfloat16)
    W16 = np.ascontiguousarray(W.astype(ml_dtypes.bfloat16))
    xT16 = np.ascontiguousarray(x.T.astype(ml_dtypes.bfloat16))  # [din, n]
    in_maps = []
    for c in range(NCORES):
        o = c * s
        # rotate node axis so this core's slab comes first
        xt_c = np.ascontiguousarray(np.roll(xT16, -o, axis=1))
        adjT_c = np.roll(adj[o:o + s, :].T, -o, axis=0)  # [n(rot), s]
        in_maps.append({
            "adjt": prep_adjt(adjT_c),
            "xt": xt_c,
            "wmat": W16,
            "att": att,
        })
    return in_maps


def kernel(x, adj, W, attn_self, attn_neigh):
    from concourse.bass_utils import run_bass_kernel_spmd

    x = np.asarray(x, dtype=np.float32)
    adj = np.asarray(adj, dtype=np.float32)
    W = np.asarray(W, dtype=np.float32)
    attn_self = np.asarray(attn_self, dtype=np.float32)
    attn_neigh = np.asarray(attn_neigh, dtype=np.float32)

    nc = build_nc()
    in_maps = make_in_maps(x, adj, W, attn_self, attn_neigh)
    res = run_bass_kernel_spmd(nc, in_maps, list(range(NCORES)))
    outs = []
    for c in range(NCORES):
        a = np.asarray(res.results[c]["out"])  # [128, 1024]
        outs.append(a.reshape(128, 8, 128).transpose(1, 0, 2).reshape(S, DOUT))
    return np.concatenate(outs, axis=0)
